# revision 6
# baseline (speedup 1.0000x reference)
"""Trainium2 Bass kernel for the nn_AaD retrieval-KNN loss (v4).

Per core: fp8 DoubleRow distance matmuls (features stationary) produce
[128, 6656] distances per batch tile in PSUM quads of 4 groups.  The only
PSUM readers are ACT and DVE, so:
  - scalar copies quads Q0/Q1 whole and Q2 in halves to SBUF as bf16,
  - vector folds Q0 max Q1 with one 2x-mode tensor_tensor, shrinks the Q2
    halves (short tail chains) and runs MAX8 + FIND_INDEX8,
  - the last group's tile is reduced by vector straight from PSUM.
Classes/segments of 8..32 columns partition all 6656 columns; the top-6
distances of a row provably live inside its core's top-8 classes, so the
host rescans the 64 winning classes per row in exact fp32, re-ranks with
lax.top_k tie-breaking, and computes the KL + dispersion loss in numpy.
"""

import numpy as np
import ml_dtypes

import concourse.mybir as mybir
import concourse.tile as tile
from concourse import bacc
from concourse.bass_utils import run_bass_kernel_spmd

B, D, C, N, K = 256, 512, 345, 50000, 5
ALPHA = 1.0
EPS = 1e-12
M = 8                   # cores
NS = N // M             # 6250 bank rows per core
G = 13                  # 512-wide column groups per core
GW = 512
NPAD = G * GW           # 6656
W = 32                  # PSUM tile innermost (segment) width
SPG = GW // W           # 16 segments per group
SV = 400                # per-m segment values: 256 + 2*64 + 16

F32 = mybir.dt.float32
BF16 = mybir.dt.bfloat16
F8 = mybir.dt.float8e4
U32 = mybir.dt.uint32
AF = mybir.ActivationFunctionType
ALU = mybir.AluOpType
DR = mybir.MatmulPerfMode.DoubleRow
AX = mybir.AxisListType.X

_CACHE: dict = {}


def _build():
    nc = bacc.Bacc("TRN2", target_bir_lowering=False, debug=False, num_devices=M)

    fbt_in = nc.dram_tensor("fbt", [128, G, 4, GW], F8, kind="ExternalInput")
    fnt_in = nc.dram_tensor("fnt", [128, 4, B], F8, kind="ExternalInput")
    out_idx = nc.dram_tensor("out_idx", [2, 128, 8], U32, kind="ExternalOutput")
    junk_out = nc.dram_tensor("junk_out", [1, 8], F32, kind="ExternalOutput")

    def flat(ap):
        return ap.rearrange("p a b -> p (a b)")

    with tile.TileContext(nc) as tc:
        with (
            tc.tile_pool(name="const", bufs=1) as constp,
            tc.tile_pool(name="small", bufs=2) as smallp,
            tc.tile_pool(name="sbt", bufs=3) as sbtp,
            tc.tile_pool(name="psum", bufs=2, space="PSUM") as psp,
        ):
            # hoist the scalar-engine act-table load into the preamble shadow
            js0 = constp.tile([1, 8], F32, tag="js0")
            nc.gpsimd.memset(js0[:], 1.0)
            js1 = constp.tile([1, 8], F32, tag="js1")
            nc.scalar.activation(js1[:], js0[:], AF.Copy)
            nc.sync.dma_start(junk_out[:], js1[:])

            fnt_sb = constp.tile([128, 4, B], F8, tag="fnt")
            nc.sync.dma_start(fnt_sb[:], fnt_in[:])

            fbt_sb = constp.tile([128, G, 4, GW], F8, tag="fbt")
            for (ga, gb), eng in [((0, 2), nc.sync), ((2, 4), nc.gpsimd),
                                  ((4, 6), nc.sync), ((6, 8), nc.gpsimd),
                                  ((12, 13), nc.sync), ((8, 10), nc.gpsimd),
                                  ((10, 12), nc.sync)]:
                eng.dma_start(fbt_sb[:, ga:gb], fbt_in[:, ga:gb])

            segv = [constp.tile([128, SV], BF16, tag=f"segv{m}",
                                name=f"segv{m}") for m in range(2)]

            def mms(pt, m, ga, gb):
                for kc in range(2):
                    for g in range(ga, gb):
                        nc.tensor.matmul(
                            pt[:, (g - ga) * SPG:(g - ga + 1) * SPG, :],
                            lhsT=fnt_sb[:, 2 * kc:2 * kc + 2,
                                        m * 128:(m + 1) * 128],
                            rhs=fbt_sb[:, g, 2 * kc:2 * kc + 2, :],
                            start=(kc == 0),
                            stop=(kc == 1),
                            perf_mode=DR,
                        )

            cq = [[None, None] for _ in range(2)]   # cq[m][q] bf16 copies
            pc = 0

            # Q0, Q1: matmul -> scalar full-quad copy
            for q, (ga, gb) in enumerate([(0, 4), (4, 8)]):
                for m in range(2):
                    pt = psp.tile([128, 4 * SPG, W], F32, tag="pp",
                                  name=f"pp{pc % 2}")
                    pc += 1
                    mms(pt, m, ga, gb)
                    c = sbtp.tile([128, 4 * SPG, W], BF16, tag=f"cq{q}_{m}",
                                  name=f"cq{q}_{m}", bufs=1)
                    nc.scalar.activation(c[:], pt[:], AF.Copy)
                    cq[m][q] = c
                    if q == 1:
                        # fold Q0 max Q1 (vector, 2x bf16), shrink on gpsimd
                        t01 = sbtp.tile([128, 2048], BF16, tag=f"t01_{m}",
                                        name=f"t01_{m}", bufs=1)
                        nc.vector.tensor_tensor(out=t01[:],
                                                in0=flat(cq[m][0]),
                                                in1=flat(cq[m][1]),
                                                op=ALU.max)
                        t8 = t01.rearrange("p (a b) -> p a b", b=8)
                        s1 = sbtp.tile([128, 256, 4], BF16, tag=f"s1_{m}",
                                       name=f"s1_{m}", bufs=1)
                        nc.vector.tensor_tensor(out=s1[:], in0=t8[:, :, 0:4],
                                                in1=t8[:, :, 4:8], op=ALU.max)
                        s2 = sbtp.tile([128, 256, 2], BF16, tag=f"s2_{m}",
                                       name=f"s2_{m}", bufs=1)
                        nc.vector.tensor_tensor(out=s2[:], in0=s1[:, :, 0:2],
                                                in1=s1[:, :, 2:4], op=ALU.max)
                        nc.vector.tensor_tensor(out=segv[m][:, 0:256],
                                                in0=flat(s2[:, :, 0:1]),
                                                in1=flat(s2[:, :, 1:2]),
                                                op=ALU.max)

            # single group 12: matmul -> vector reduce direct from PSUM
            for m in range(2):
                pt = psp.tile([128, 4 * SPG, W], F32, tag="pp",
                              name=f"pp{pc % 2}")
                pc += 1
                mms(pt, m, 12, G)
                nc.vector.tensor_reduce(out=segv[m][:, 384:400],
                                        in_=pt[:, 0:SPG, :], axis=AX,
                                        op=ALU.max)

            # Q2 (groups 8-11): matmul -> scalar half copies -> vector shrink
            for m in range(2):
                pt = psp.tile([128, 4 * SPG, W], F32, tag="pp",
                              name=f"pp{pc % 2}")
                pc += 1
                mms(pt, m, 8, 12)
                for h in range(2):
                    ch = sbtp.tile([128, 2 * SPG, W], BF16, tag=f"ch{h}",
                                   name=f"ch{m}_{h}")
                    nc.scalar.activation(ch[:], pt[:, h * 2 * SPG:
                                                   (h + 1) * 2 * SPG, :],
                                         AF.Copy)
                    h8 = ch.rearrange("p a b -> p (a b)").rearrange(
                        "p (a b) -> p a b", b=8)
                    a = smallp.tile([128, 128, 4], BF16, tag="ha",
                                    name=f"ha{m}_{h}")
                    nc.vector.tensor_tensor(out=a[:], in0=h8[:, :, 0:4],
                                            in1=h8[:, :, 4:8], op=ALU.max)
                    b = smallp.tile([128, 128, 2], BF16, tag="hb",
                                    name=f"hb{m}_{h}")
                    nc.vector.tensor_tensor(out=b[:], in0=a[:, :, 0:2],
                                            in1=a[:, :, 2:4], op=ALU.max)
                    b4 = b.rearrange("p a b -> p (a b)").rearrange(
                        "p (a b) -> p a b", b=4)
                    nc.vector.tensor_reduce(
                        out=segv[m][:, 256 + 64 * h:256 + 64 * h + 64],
                        in_=b4[:], axis=AX, op=ALU.max)

                mx8 = smallp.tile([128, 8], BF16, tag=f"mx8_{m}",
                                  name=f"mx8_{m}")
                nc.vector.max(out=mx8[:], in_=segv[m][:])
                sel8 = smallp.tile([128, 8], U32, tag=f"sel8_{m}",
                                   name=f"sel8_{m}")
                nc.vector.max_index(out=sel8[:], in_max=mx8[:],
                                    in_values=segv[m][:])
                nc.sync.dma_start(out_idx[m], sel8[:])

    nc.compile()
    return nc


def _get_nc():
    if "nc" not in _CACHE:
        _CACHE["nc"] = _build()
    return _CACHE["nc"]


def _prep(features, predictions, fea_bank, score_bank, trg_idx):
    feat = np.asarray(features, dtype=np.float32)
    pred = np.asarray(predictions, dtype=np.float32)
    fb = np.array(fea_bank, dtype=np.float32)
    sb = np.array(score_bank, dtype=np.float32)
    trg = np.asarray(trg_idx).astype(np.int64)

    x = pred - pred.max(axis=1, keepdims=True)
    e = np.exp(x)
    p = e / e.sum(axis=1, keepdims=True)

    nrm = np.sqrt((feat * feat).sum(axis=1, keepdims=True))
    fn = feat / np.maximum(nrm, EPS)

    fb[trg] = fn
    sb[trg] = p

    fnt = np.ascontiguousarray(
        fn.T.reshape(4, 128, B).transpose(1, 0, 2)).astype(ml_dtypes.float8_e4m3)

    in_maps = []
    for c in range(M):
        slabT = np.zeros((D, NPAD), dtype=np.float32)
        slabT[:, :NS] = fb[c * NS:(c + 1) * NS].T
        fbt = np.ascontiguousarray(
            slabT.reshape(4, 128, G, GW).transpose(1, 2, 0, 3)
        ).astype(ml_dtypes.float8_e4m3)
        in_maps.append({"fbt": fbt, "fnt": fnt})
    return in_maps, fn, fb, sb, p


def _seg_cols():
    """Map each of the SV segment ids to its (padded) column list [SV, 32]."""
    cols = np.full((SV, 32), NPAD, dtype=np.int64)   # NPAD == invalid
    for sid in range(256):
        pis = np.arange(8 * sid, 8 * sid + 8)
        for j, pi in enumerate(pis):
            for q in range(2):
                cols[sid, 2 * j + q] = (4 * q + pi // 512) * 512 + pi % 512
    for sid in range(256, 384):
        h = (sid - 256) // 64
        s = (sid - 256) % 64
        fs = np.arange(16 * s, 16 * s + 16)
        cols[sid, :16] = (8 + 2 * h + fs // 512) * 512 + fs % 512
    for sid in range(384, SV):
        s = sid - 384
        cols[sid, :] = 6144 + 32 * s + np.arange(32)
    return cols


_SEG_COLS = _seg_cols()


def _merge(results, fn, fb, sb, p):
    gls, vas = [], []
    for c in range(M):
        sel = results[c]["out_idx"].reshape(B, 8).astype(np.int64)
        cols = _SEG_COLS[sel].reshape(B, 8 * 32)    # core-local padded cols
        valid = cols < NS
        gls.append(c * NS + np.minimum(cols, NS - 1))
        vas.append(valid)
    gi = np.concatenate(gls, axis=1)                # [B, 2048]
    va = np.concatenate(vas, axis=1)

    V = np.einsum("bkd,bd->bk", fb[gi], fn, optimize=True).astype(np.float32)
    V = np.where(va, V, -np.inf)

    # lax.top_k order: value desc, ties -> lowest original index
    order = np.lexsort((gi, -V.astype(np.float64)), axis=-1)

    sel_gi = np.empty((B, K), dtype=np.int64)
    for b in range(B):
        got = 0
        prev = -1
        for pos in order[b]:
            g = gi[b, pos]
            if g == prev:
                continue
            prev = g
            if got > 0:
                sel_gi[b, got - 1] = g
            got += 1
            if got == K + 1:
                break

    sbs = sb[sel_gi].astype(np.float64)             # [B, K, C]
    h = (sbs * np.log(sbs)).sum(-1)
    q = np.einsum("bkc,bc->bk", sbs, p.astype(np.float64))
    kl = (h - q).sum(-1).mean()

    ps = p.astype(np.float64)
    disp = ((ps.sum(0) ** 2).sum() - (ps * ps).sum()) / B
    return np.float32(kl + ALPHA * disp)


def run(inputs, trace=False):
    nc = _get_nc()
    in_maps, fn, fb, sb, p = _prep(**inputs)
    res = run_bass_kernel_spmd(nc, in_maps, list(range(M)), trace=trace)
    return _merge(res.results, fn, fb, sb, p), res


def kernel(features, predictions, fea_bank, score_bank, trg_idx):
    loss, _ = run(
        dict(
            features=features,
            predictions=predictions,
            fea_bank=fea_bank,
            score_bank=score_bank,
            trg_idx=trg_idx,
        )
    )
    return loss


# revision 7
# speedup vs baseline: 1.1044x; 1.1044x over previous
"""Trainium2 Bass kernel for the nn_AaD retrieval-KNN loss (v4).

Per core: fp8 DoubleRow distance matmuls (features stationary) produce
[128, 6656] distances per batch tile in PSUM quads of 4 groups.  The only
PSUM readers are ACT and DVE, so:
  - scalar copies quads Q0/Q1 whole and Q2 in halves to SBUF as bf16,
  - vector folds Q0 max Q1 with one 2x-mode tensor_tensor, shrinks the Q2
    halves (short tail chains) and runs MAX8 + FIND_INDEX8,
  - the last group's tile is reduced by vector straight from PSUM.
Classes/segments of 8..32 columns partition all 6656 columns; the top-6
distances of a row provably live inside its core's top-8 classes, so the
host rescans the 64 winning classes per row in exact fp32, re-ranks with
lax.top_k tie-breaking, and computes the KL + dispersion loss in numpy.
"""

import numpy as np
import ml_dtypes

import concourse.mybir as mybir
import concourse.tile as tile
from concourse import bacc
from concourse.bass_utils import run_bass_kernel_spmd

B, D, C, N, K = 256, 512, 345, 50000, 5
ALPHA = 1.0
EPS = 1e-12
M = 8                   # cores
NS = N // M             # 6250 bank rows per core
G = 13                  # 512-wide column groups per core
GW = 512
NPAD = G * GW           # 6656
W = 32                  # PSUM tile innermost (segment) width
SPG = GW // W           # 16 segments per group
SV = 400                # per-m segment values: 256 + 2*64 + 16

F32 = mybir.dt.float32
BF16 = mybir.dt.bfloat16
F8 = mybir.dt.float8e4
U32 = mybir.dt.uint32
AF = mybir.ActivationFunctionType
ALU = mybir.AluOpType
DR = mybir.MatmulPerfMode.DoubleRow
AX = mybir.AxisListType.X

_CACHE: dict = {}


def _build():
    nc = bacc.Bacc("TRN2", target_bir_lowering=False, debug=False, num_devices=M)

    fbt_in = nc.dram_tensor("fbt", [128, G, 4, GW], F8, kind="ExternalInput")
    fnt_in = nc.dram_tensor("fnt", [128, 4, B], F8, kind="ExternalInput")
    out_idx = nc.dram_tensor("out_idx", [2, 128, 8], U32, kind="ExternalOutput")

    def flat(ap):
        return ap.rearrange("p a b -> p (a b)")

    with tile.TileContext(nc) as tc:
        with (
            tc.tile_pool(name="const", bufs=1) as constp,
            tc.tile_pool(name="small", bufs=2) as smallp,
            tc.tile_pool(name="sbt", bufs=3) as sbtp,
            tc.tile_pool(name="psum", bufs=2, space="PSUM") as psp,
        ):
            fnt_sb = constp.tile([128, 4, B], F8, tag="fnt")
            nc.sync.dma_start(fnt_sb[:], fnt_in[:])

            fbt_sb = constp.tile([128, G, 4, GW], F8, tag="fbt")
            for (ga, gb) in [(0, 2), (2, 4), (4, 6), (6, 8),
                             (8, 10), (10, 12), (12, 13)]:
                nc.sync.dma_start(fbt_sb[:, ga:gb], fbt_in[:, ga:gb])

            segv = [constp.tile([128, SV], BF16, tag=f"segv{m}",
                                name=f"segv{m}") for m in range(2)]

            def mms(pt, m, ga, gb):
                for kc in range(2):
                    for g in range(ga, gb):
                        nc.tensor.matmul(
                            pt[:, (g - ga) * SPG:(g - ga + 1) * SPG, :],
                            lhsT=fnt_sb[:, 2 * kc:2 * kc + 2,
                                        m * 128:(m + 1) * 128],
                            rhs=fbt_sb[:, g, 2 * kc:2 * kc + 2, :],
                            start=(kc == 0),
                            stop=(kc == 1),
                            perf_mode=DR,
                        )

            cq = [[None, None] for _ in range(2)]   # cq[m][q] bf16 copies
            pc = 0

            # Q0, Q1: matmul -> scalar full-quad copy
            for q, (ga, gb) in enumerate([(0, 4), (4, 8)]):
                for m in range(2):
                    pt = psp.tile([128, 4 * SPG, W], F32, tag="pp",
                                  name=f"pp{pc % 2}")
                    pc += 1
                    mms(pt, m, ga, gb)
                    c = sbtp.tile([128, 4 * SPG, W], BF16, tag=f"cq{q}_{m}",
                                  name=f"cq{q}_{m}", bufs=1)
                    nc.scalar.activation(c[:], pt[:], AF.Copy)
                    cq[m][q] = c
                    if q == 1:
                        # fold Q0 max Q1 (vector, 2x bf16), shrink on gpsimd
                        t01 = sbtp.tile([128, 2048], BF16, tag=f"t01_{m}",
                                        name=f"t01_{m}", bufs=1)
                        nc.vector.tensor_tensor(out=t01[:],
                                                in0=flat(cq[m][0]),
                                                in1=flat(cq[m][1]),
                                                op=ALU.max)
                        t8 = t01.rearrange("p (a b) -> p a b", b=8)
                        s1 = sbtp.tile([128, 256, 4], BF16, tag=f"s1_{m}",
                                       name=f"s1_{m}", bufs=1)
                        nc.vector.tensor_tensor(out=s1[:], in0=t8[:, :, 0:4],
                                                in1=t8[:, :, 4:8], op=ALU.max)
                        s2 = sbtp.tile([128, 256, 2], BF16, tag=f"s2_{m}",
                                       name=f"s2_{m}", bufs=1)
                        nc.vector.tensor_tensor(out=s2[:], in0=s1[:, :, 0:2],
                                                in1=s1[:, :, 2:4], op=ALU.max)
                        nc.vector.tensor_tensor(out=segv[m][:, 0:256],
                                                in0=flat(s2[:, :, 0:1]),
                                                in1=flat(s2[:, :, 1:2]),
                                                op=ALU.max)

            # Q2 (groups 8-11): matmul -> scalar half copies -> vector shrink
            for m in range(2):
                pt = psp.tile([128, 4 * SPG, W], F32, tag="pp",
                              name=f"pp{pc % 2}")
                pc += 1
                mms(pt, m, 8, 12)
                for h in range(2):
                    ch = sbtp.tile([128, 2 * SPG, W], BF16, tag=f"ch{h}",
                                   name=f"ch{m}_{h}")
                    nc.scalar.activation(ch[:], pt[:, h * 2 * SPG:
                                                   (h + 1) * 2 * SPG, :],
                                         AF.Copy)
                    h8 = ch.rearrange("p a b -> p (a b)").rearrange(
                        "p (a b) -> p a b", b=8)
                    a = smallp.tile([128, 128, 4], BF16, tag="ha",
                                    name=f"ha{m}_{h}")
                    nc.vector.tensor_tensor(out=a[:], in0=h8[:, :, 0:4],
                                            in1=h8[:, :, 4:8], op=ALU.max)
                    b = smallp.tile([128, 128, 2], BF16, tag="hb",
                                    name=f"hb{m}_{h}")
                    nc.vector.tensor_tensor(out=b[:], in0=a[:, :, 0:2],
                                            in1=a[:, :, 2:4], op=ALU.max)
                    b4 = b.rearrange("p a b -> p (a b)").rearrange(
                        "p (a b) -> p a b", b=4)
                    nc.vector.tensor_reduce(
                        out=segv[m][:, 256 + 64 * h:256 + 64 * h + 64],
                        in_=b4[:], axis=AX, op=ALU.max)

            # single group 12: matmul -> vector reduce direct from PSUM
            for m in range(2):
                pt = psp.tile([128, 4 * SPG, W], F32, tag="pp",
                              name=f"pp{pc % 2}")
                pc += 1
                mms(pt, m, 12, G)
                nc.vector.tensor_reduce(out=segv[m][:, 384:400],
                                        in_=pt[:, 0:SPG, :], axis=AX,
                                        op=ALU.max)
                mx8 = smallp.tile([128, 8], BF16, tag=f"mx8_{m}",
                                  name=f"mx8_{m}")
                nc.vector.max(out=mx8[:], in_=segv[m][:])
                sel8 = smallp.tile([128, 8], U32, tag=f"sel8_{m}",
                                   name=f"sel8_{m}")
                nc.vector.max_index(out=sel8[:], in_max=mx8[:],
                                    in_values=segv[m][:])
                nc.sync.dma_start(out_idx[m], sel8[:])


    nc.compile()
    return nc


def _get_nc():
    if "nc" not in _CACHE:
        _CACHE["nc"] = _build()
    return _CACHE["nc"]


def _prep(features, predictions, fea_bank, score_bank, trg_idx):
    feat = np.asarray(features, dtype=np.float32)
    pred = np.asarray(predictions, dtype=np.float32)
    fb = np.array(fea_bank, dtype=np.float32)
    sb = np.array(score_bank, dtype=np.float32)
    trg = np.asarray(trg_idx).astype(np.int64)

    x = pred - pred.max(axis=1, keepdims=True)
    e = np.exp(x)
    p = e / e.sum(axis=1, keepdims=True)

    nrm = np.sqrt((feat * feat).sum(axis=1, keepdims=True))
    fn = feat / np.maximum(nrm, EPS)

    fb[trg] = fn
    sb[trg] = p

    fnt = np.ascontiguousarray(
        fn.T.reshape(4, 128, B).transpose(1, 0, 2)).astype(ml_dtypes.float8_e4m3)

    in_maps = []
    for c in range(M):
        slabT = np.zeros((D, NPAD), dtype=np.float32)
        slabT[:, :NS] = fb[c * NS:(c + 1) * NS].T
        fbt = np.ascontiguousarray(
            slabT.reshape(4, 128, G, GW).transpose(1, 2, 0, 3)
        ).astype(ml_dtypes.float8_e4m3)
        in_maps.append({"fbt": fbt, "fnt": fnt})
    return in_maps, fn, fb, sb, p


def _seg_cols():
    """Map each of the SV segment ids to its (padded) column list [SV, 32]."""
    cols = np.full((SV, 32), NPAD, dtype=np.int64)   # NPAD == invalid
    for sid in range(256):
        pis = np.arange(8 * sid, 8 * sid + 8)
        for j, pi in enumerate(pis):
            for q in range(2):
                cols[sid, 2 * j + q] = (4 * q + pi // 512) * 512 + pi % 512
    for sid in range(256, 384):
        h = (sid - 256) // 64
        s = (sid - 256) % 64
        fs = np.arange(16 * s, 16 * s + 16)
        cols[sid, :16] = (8 + 2 * h + fs // 512) * 512 + fs % 512
    for sid in range(384, SV):
        s = sid - 384
        cols[sid, :] = 6144 + 32 * s + np.arange(32)
    return cols


_SEG_COLS = _seg_cols()


def _merge(results, fn, fb, sb, p):
    gls, vas = [], []
    for c in range(M):
        sel = results[c]["out_idx"].reshape(B, 8).astype(np.int64)
        cols = _SEG_COLS[sel].reshape(B, 8 * 32)    # core-local padded cols
        valid = cols < NS
        gls.append(c * NS + np.minimum(cols, NS - 1))
        vas.append(valid)
    gi = np.concatenate(gls, axis=1)                # [B, 2048]
    va = np.concatenate(vas, axis=1)

    V = np.einsum("bkd,bd->bk", fb[gi], fn, optimize=True).astype(np.float32)
    V = np.where(va, V, -np.inf)

    # lax.top_k order: value desc, ties -> lowest original index
    order = np.lexsort((gi, -V.astype(np.float64)), axis=-1)

    sel_gi = np.empty((B, K), dtype=np.int64)
    for b in range(B):
        got = 0
        prev = -1
        for pos in order[b]:
            g = gi[b, pos]
            if g == prev:
                continue
            prev = g
            if got > 0:
                sel_gi[b, got - 1] = g
            got += 1
            if got == K + 1:
                break

    sbs = sb[sel_gi].astype(np.float64)             # [B, K, C]
    h = (sbs * np.log(sbs)).sum(-1)
    q = np.einsum("bkc,bc->bk", sbs, p.astype(np.float64))
    kl = (h - q).sum(-1).mean()

    ps = p.astype(np.float64)
    disp = ((ps.sum(0) ** 2).sum() - (ps * ps).sum()) / B
    return np.float32(kl + ALPHA * disp)


def run(inputs, trace=False):
    nc = _get_nc()
    in_maps, fn, fb, sb, p = _prep(**inputs)
    res = run_bass_kernel_spmd(nc, in_maps, list(range(M)), trace=trace)
    return _merge(res.results, fn, fb, sb, p), res


def kernel(features, predictions, fea_bank, score_bank, trg_idx):
    loss, _ = run(
        dict(
            features=features,
            predictions=predictions,
            fea_bank=fea_bank,
            score_bank=score_bank,
            trg_idx=trg_idx,
        )
    )
    return loss
